# revision 8
# baseline (speedup 1.0000x reference)
"""GCN encoder (2x GCNConv + BatchNorm + PReLU) on 8 Trainium2 NeuronCores.

Full inputs in, full outputs out. v2 design:
  - nodes sharded contiguously across 8 cores (12500 real rows + pad -> 12544),
  - v' = dinv_src * (h @ W) computed locally per core, cast to fp16,
  - AllGather of the fp16 v' table (halo = everything on a random graph),
  - per-edge messages fetched with gpsimd.dma_gather across 4 SWDGE queues
    (descriptor prep parallelizes across queues; this was the v1 bottleneck),
  - segment-sum done ON THE TENSOR ENGINE: edges are sorted by dst tile-pair,
    each 128-edge chunk is multiplied by a DVE-built one-hot selector
    [128 edges x 256 dst slots] and accumulated into a PSUM tile per
    dst-pair -- no dma_scatter_add, no HBM accumulator round trip,
  - conv = dinv_dst * (psum + dinv_dst * v) adds the self-loop analytically,
  - BN stats via free-dim reduce in [feat, rows] layout, stats AllReduce,
    fused BN+PReLU via two ACT Relu passes + one DVE scalar_tensor_tensor.

Stream layout (identical structure on all 8 cores -- SPMD): edges sorted by
(slab, dst-pair) where slab = vfull_row // 32768 (dma_gather idxs are int16,
so each call reads one <=32768-row slab of the AllGathered table). Each
(slab, pair) segment is padded to a 128 multiple with pointers to a known
all-zero table row, and segment sizes are maxed over cores so the compiled
chunk->pair structure is core-independent. Bands (slabs) are padded to the
2048-idx call size, so call k always covers stream chunks [16k, 16k+16).
"""

import numpy as np

import concourse.bass as bass
import concourse.bacc as bacc
import concourse.tile as tile
from concourse import mybir
from concourse import bass_utils
from concourse.masks import make_identity
from concourse.bass_interp import get_hw_module

F32 = mybir.dt.float32
F16 = mybir.dt.float16
I16 = mybir.dt.int16
EPS = 1e-5
NB = 8          # cores
D = 128
SLAB = 32768    # int16 index range per gather call
CALLSZ = 4096   # max idxs per dma_gather call (32 chunks)
PAIR = 256      # dst slots per psum accumulation tile


# ---------------------------------------------------------------- host side


def preprocess(x: np.ndarray, edge_index: np.ndarray):
    N = x.shape[0]
    nsh = (N + NB - 1) // NB                     # 12500 real rows per shard
    SH = ((nsh + 1 + 127) // 128) * 128          # 12544 padded rows
    NT = SH // 128                               # 98 tiles
    NP = (SH + PAIR - 1) // PAIR                 # 49 dst pairs
    NSLAB = (NB * SH + SLAB - 1) // SLAB         # 4 slabs over vfull

    src = edge_index[0]
    dst = edge_index[1]
    deg = np.bincount(dst, minlength=N) + 1      # +1: self-loop
    dinv = (1.0 / np.sqrt(deg.astype(np.float64))).astype(np.float32)

    # relative index of a guaranteed all-zero vloc row inside each slab
    zrel = []
    for j in range(NSLAB):
        base = j * SLAB
        zr = None
        for b in range(NB):
            z0 = b * SH + nsh                    # first zero row of block b
            if base <= z0 < min(base + SLAB, NB * SH):
                zr = z0 - base
                break
        assert zr is not None
        zrel.append(zr)

    # per-core edge lists sorted by (slab, dst pair)
    per_core = []
    counts = np.zeros((NB, NSLAB, NP), np.int64)
    for c in range(NB):
        m = (dst // nsh) == c
        gs = src[m]
        sl = (dst[m] - c * nsh).astype(np.int64)
        vrow = (gs // nsh) * SH + (gs % nsh)
        slab = vrow // SLAB
        P = sl // PAIR
        order = np.lexsort((P, slab))
        vrow, sl, slab, P = vrow[order], sl[order], slab[order], P[order]
        per_core.append((vrow, sl, slab, P))
        counts[c] = np.bincount(slab * NP + P,
                                minlength=NSLAB * NP).reshape(NSLAB, NP)

    K = np.ceil(counts.max(axis=0) / 128).astype(np.int64)   # [NSLAB, NP]
    seg_rows = K * 128

    # band-major stream layout; each band padded to a CALLSZ multiple
    band_rows = seg_rows.sum(axis=1)
    band_cap = ((band_rows + 127) // 128) * 128
    band_cap = np.maximum(band_cap, 128)
    band_off = np.concatenate([[0], np.cumsum(band_cap)])
    S = int(band_off[-1])                        # total stream rows
    seg_off = np.zeros((NSLAB, NP), np.int64)
    for j in range(NSLAB):
        seg_off[j] = band_off[j] + np.concatenate(
            [[0], np.cumsum(seg_rows[j])[:-1]])

    # fill per-core index + dst tables
    gidx = np.zeros((NB, 128, S // 16), np.int16)
    dst32 = np.zeros((NB, 128, S // 128), np.float32)
    dinvrow = np.zeros((NB, 128, SH), np.float16)
    dinv_cols = np.zeros((NB, 128, NT), np.float32)
    x_sh = np.zeros((NB, SH, D), np.float32)
    for c in range(NB):
        g = np.zeros(S, np.int64)
        for j in range(NSLAB):
            g[band_off[j] : band_off[j + 1]] = zrel[j]
        dv = np.full(S, 300.0, np.float64)
        vrow, sl, slab, P = per_core[c]
        # slot position for each edge: segment start + rank within segment
        segid = slab * NP + P
        seg_start = seg_off.reshape(-1)[segid]
        # edges are sorted by segid, so rank = index - first index of segid
        first = np.searchsorted(segid, segid, side="left")
        pos = seg_start + (np.arange(len(segid)) - first)
        g[pos] = vrow - slab * SLAB
        dv[pos] = sl % PAIR
        gidx[c] = np.tile(
            np.ascontiguousarray(g.astype(np.int16).reshape(S // 16, 16).T),
            (8, 1))
        dst32[c] = np.ascontiguousarray(
            dv.astype(np.float32).reshape(S // 128, 128).T)

        lo = c * nsh
        hi = min(lo + nsh, N)
        dloc = np.zeros(SH, np.float32)
        dloc[: hi - lo] = dinv[lo:hi]
        dinvrow[c] = np.tile(dloc.astype(np.float16)[None, :], (128, 1))
        dinv_cols[c] = dloc.reshape(NT, 128).T
        x_sh[c, : hi - lo] = x[lo:hi]

    return dict(
        N=N, nsh=nsh, SH=SH, S=S,
        K=tuple(map(tuple, K)), band_off=tuple(int(b) for b in band_off),
        seg_off=tuple(map(tuple, seg_off)),
        gidx=gidx, dst32=dst32, dinvrow=dinvrow, dinv_cols=dinv_cols,
        x_sh=x_sh,
    )


# -------------------------------------------------------------- device side


def build_kernel(N: int, SH: int, S: int, K, band_off, seg_off):
    nc = bacc.Bacc("TRN2", target_bir_lowering=False, debug=False,
                   num_devices=NB, num_swdge_queues=4)
    rg = [list(range(NB))]
    NT = SH // 128
    NP = SH // PAIR
    NSLAB = len(band_off) - 1
    chunks = [(o, min(512, SH - o)) for o in range(0, SH, 512)]

    x_in = nc.dram_tensor("x", [SH, D], F32, kind="ExternalInput")
    gidx_in = nc.dram_tensor("gidx", [128, S // 16], I16,
                             kind="ExternalInput")
    dst_in = nc.dram_tensor("dst32", [128, S // 128], F32,
                            kind="ExternalInput")
    dnr_in = nc.dram_tensor("dinvrow", [128, SH], F16, kind="ExternalInput")
    dinv_in = nc.dram_tensor("dinv_cols", [128, NT], F32,
                             kind="ExternalInput")
    iota_in = nc.dram_tensor("iota256", [128, PAIR], F16,
                             kind="ExternalInput")
    w_in = [nc.dram_tensor(f"w{l}", [D, D], F32, kind="ExternalInput")
            for l in range(2)]
    gam_in = [nc.dram_tensor(f"gamma{l}", [D, 1], F32, kind="ExternalInput")
              for l in range(2)]
    bet_in = [nc.dram_tensor(f"beta{l}", [D, 1], F32, kind="ExternalInput")
              for l in range(2)]
    a_in = [nc.dram_tensor(f"a{l}", [D, 1], F32, kind="ExternalInput")
            for l in range(2)]
    out_t = nc.dram_tensor("out", [SH, D], F32, kind="ExternalOutput")

    vloc = nc.dram_tensor("vloc", [SH, D], F16)
    vfull = nc.dram_tensor("vfull", [NB * SH, D], F16, addr_space="Shared")
    stats_in = nc.dram_tensor("stats_in", [D, 2], F32)
    stats_out = nc.dram_tensor("stats_out", [D, 2], F32, addr_space="Shared")

    out_r = out_t.ap().rearrange("(t p) f -> t p f", p=128)
    x_r = x_in.ap().rearrange("(t p) f -> t p f", p=128)
    vloc_r = vloc.ap().rearrange("(t p) f -> t p f", p=128)


    with tile.TileContext(nc) as tc:
        with (
            tc.tile_pool(name="pers", bufs=1) as PE_,
            tc.tile_pool(name="act", bufs=1) as PA_,
            tc.tile_pool(name="msg", bufs=7) as PM,
            tc.tile_pool(name="sel", bufs=4) as PSL,
            tc.tile_pool(name="work", bufs=3) as PW,
            tc.tile_pool(name="small", bufs=2) as PS,
            tc.tile_pool(name="psP", bufs=3, space="PSUM") as PP,
            tc.tile_pool(name="psV", bufs=1, space="PSUM") as PV,
            tc.tile_pool(name="psA", bufs=1, space="PSUM") as PAP,
            tc.tile_pool(name="psT", bufs=1, space="PSUM") as PT,
        ):
            ident = PE_.tile([128, 128], F32, tag="ident")
            make_identity(nc, ident[:])
            ident16 = PE_.tile([128, 128], F16, tag="ident16")
            nc.vector.tensor_copy(ident16[:], ident[:])
            gidx_sb = PE_.tile([128, S // 16], I16, tag="gidx")
            nc.sync.dma_start(gidx_sb[:], gidx_in.ap())
            dst_sb = PE_.tile([128, S // 128], F32, tag="dst32")
            nc.sync.dma_start(dst_sb[:], dst_in.ap())
            dnr_sb = PE_.tile([128, SH], F16, tag="dinvrow")
            nc.sync.dma_start(dnr_sb[:], dnr_in.ap())
            dinv_sb = PE_.tile([128, NT], F32, tag="dinv")
            nc.sync.dma_start(dinv_sb[:], dinv_in.ap())
            iota_sb = PE_.tile([128, PAIR], F16, tag="iota")
            nc.sync.dma_start(iota_sb[:], iota_in.ap())
            w_sb, gam_sb, bet_sb, a_sb = [], [], [], []
            for l in range(2):
                w_sb.append(PE_.tile([128, 128], F32, tag=f"w{l}",
                                     name=f"w{l}_sb"))
                nc.sync.dma_start(w_sb[l][:], w_in[l].ap())
                gam_sb.append(PE_.tile([128, 1], F32, tag=f"g{l}", name=f"g{l}_sb"))
                nc.sync.dma_start(gam_sb[l][:], gam_in[l].ap())
                bet_sb.append(PE_.tile([128, 1], F32, tag=f"b{l}", name=f"b{l}_sb"))
                nc.sync.dma_start(bet_sb[l][:], bet_in[l].ap())
                a_sb.append(PE_.tile([128, 1], F32, tag=f"a{l}", name=f"a{l}_sb"))
                nc.sync.dma_start(a_sb[l][:], a_in[l].ap())
            zero_sb = PE_.tile([128, 128], F32, tag="zero")
            nc.vector.memset(zero_sb[:], 0.0)
            eps_sb = PE_.tile([128, 1], F32, tag="eps")
            nc.vector.memset(eps_sb[:], EPS)

            actT = PA_.tile([128, SH], F32, tag="actT")  # h as [feat, rows]

            # ---- load x, transpose into actT
            for t in range(NT):
                xt = PW.tile([128, 128], F32, tag="xt")
                nc.sync.dma_start(xt[:], x_r[t])
                tp = PT.tile([128, 128], F32, tag="tp")
                nc.tensor.transpose(out=tp[:], in_=xt[:], identity=ident[:])
                nc.vector.tensor_copy(actT[:, 128 * t : 128 * (t + 1)], tp[:])

            for l in range(2):
                # ---- v' table: vloc[t] = f16(dinv_src * (W.T @ actT)[.,t].T)
                for (o, cw) in chunks:
                    vp = PAP.tile([128, 512], F32, tag="vp")
                    nc.tensor.matmul(out=vp[:, :cw], lhsT=w_sb[l][:],
                                     rhs=actT[:, o : o + cw],
                                     start=True, stop=True)
                    vt = PW.tile([128, 512], F16, tag="vt")
                    nc.vector.tensor_copy(vt[:, :cw], vp[:, :cw])
                    for s in range(0, cw, 128):
                        t = (o + s) // 128
                        tp = PT.tile([128, 128], F16, tag="tph")
                        nc.tensor.transpose(out=tp[:], in_=vt[:, s : s + 128],
                                            identity=ident16[:])
                        vv = PW.tile([128, 128], F16, tag="vv")
                        nc.vector.tensor_scalar(
                            vv[:], tp[:], dinv_sb[:, t : t + 1], None,
                            op0=mybir.AluOpType.mult)
                        nc.sync.dma_start(vloc_r[t], vv[:])

                # ---- halo exchange: AllGather the fp16 table
                nc.gpsimd.collective_compute(
                    "AllGather", mybir.AluOpType.bypass, replica_groups=rg,
                    ins=[vloc.ap().opt()], outs=[vfull.ap().opt()])

                # ---- gather + selector-matmul scatter, pair by pair
                call_tiles = [None] * (S // 128)
                band_next = [band_off[j] for j in range(NSLAB)]

                def issue_through(P):
                    # gather everything needed for pairs <= P, per band
                    for j in range(NSLAB):
                        limit = seg_off[j][P] + 128 * K[j][P]
                        while (band_next[j] < band_off[j + 1]
                               and band_next[j] < limit):
                            pos = band_next[j]
                            sz = min(CALLSZ, band_off[j + 1] - pos)
                            k = pos // 128          # chunk index of call
                            mt = PM.tile([128, CALLSZ // 128, 128], F16,
                                         tag="mt", name=f"mt{l}_{k}")
                            nc.gpsimd.dma_gather(
                                out_ap=mt[:, : sz // 128, :],
                                in_ap=vfull.ap()[j * SLAB :
                                                 min((j + 1) * SLAB,
                                                     NB * SH), :],
                                idxs_ap=gidx_sb[:, pos // 16 :
                                                (pos + sz) // 16],
                                num_idxs=sz, num_idxs_reg=sz,
                                elem_size=D, single_packet=False,
                                queue_num=qn[0] % 4)
                            qn[0] += 1
                            for cc in range(sz // 128):
                                call_tiles[k + cc] = (mt, cc)
                            band_next[j] += sz

                qn = [0]
                for P in range(NP):
                    issue_through(min(P + 1, NP - 1))
                    pcw = min(PAIR, SH - P * PAIR)
                    ps = PP.tile([128, PAIR], F32, tag="ps")
                    # chunk list for this pair across bands
                    pchunks = []
                    for j in range(NSLAB):
                        g0 = seg_off[j][P] // 128
                        pchunks.append((g0, K[j][P]))
                    total = sum(k for _, k in pchunks)
                    done = 0
                    for (g0, kk) in pchunks:
                        if kk == 0:
                            continue
                        for i in range(kk):
                            g = g0 + i
                            sel = PSL.tile([128, PAIR], F16, tag="sel",
                                           name=f"sel{P}_{i}")
                            nc.vector.tensor_scalar(
                                sel[:], iota_sb[:], dst_sb[:, g : g + 1],
                                None, op0=mybir.AluOpType.is_equal)
                            mt, cc = call_tiles[g]
                            nc.tensor.matmul(
                                out=ps[:, :pcw],
                                lhsT=mt[:, cc, :],
                                rhs=sel[:, :pcw],
                                start=(done == 0), stop=(done == total - 1))
                            done += 1
                    # ---- conv = dinv*(ps + dinv*v); v recomputed on PE
                    vq = PV.tile([128, PAIR], F32, tag="vq")
                    nc.tensor.matmul(out=vq[:, :pcw], lhsT=w_sb[l][:],
                                     rhs=actT[:, P * PAIR : P * PAIR + pcw],
                                     start=True, stop=True)
                    dsl = dnr_sb[:, P * PAIR : P * PAIR + pcw]
                    t1 = PW.tile([128, PAIR], F32, tag="t1")
                    nc.vector.tensor_tensor(out=t1[:, :pcw], in0=vq[:, :pcw],
                                            in1=dsl, op=mybir.AluOpType.mult)
                    t2 = PW.tile([128, PAIR], F32, tag="t2")
                    nc.vector.tensor_tensor(out=t2[:, :pcw], in0=ps[:, :pcw],
                                            in1=t1[:, :pcw],
                                            op=mybir.AluOpType.add)
                    nc.vector.tensor_tensor(
                        out=actT[:, P * PAIR : P * PAIR + pcw],
                        in0=t2[:, :pcw], in1=dsl, op=mybir.AluOpType.mult)

                # ---- BN stats (biased, over the real N rows; pad rows are 0)
                nk = len(chunks)
                sumc = PS.tile([128, nk], F32, tag="sumc")
                sqc = PS.tile([128, nk], F32, tag="sqc")
                for k, (o, cw) in enumerate(chunks):
                    nc.vector.tensor_reduce(
                        out=sumc[:, k : k + 1], in_=actT[:, o : o + cw],
                        axis=mybir.AxisListType.X, op=mybir.AluOpType.add)
                    sq = PW.tile([128, 512], F32, tag="sq")
                    nc.scalar.activation(
                        out=sq[:, :cw], in_=actT[:, o : o + cw],
                        func=mybir.ActivationFunctionType.Square,
                        bias=zero_sb[:, 0:1],
                        accum_out=sqc[:, k : k + 1])
                stats_sb = PS.tile([128, 2], F32, tag="stats")
                nc.vector.tensor_reduce(out=stats_sb[:, 0:1], in_=sumc[:],
                                        axis=mybir.AxisListType.X,
                                        op=mybir.AluOpType.add)
                nc.vector.tensor_reduce(out=stats_sb[:, 1:2], in_=sqc[:],
                                        axis=mybir.AxisListType.X,
                                        op=mybir.AluOpType.add)
                nc.sync.dma_start(stats_in.ap(), stats_sb[:])
                nc.gpsimd.collective_compute(
                    "AllReduce", mybir.AluOpType.add, replica_groups=rg,
                    ins=[stats_in.ap().opt()], outs=[stats_out.ap().opt()])
                stats2 = PS.tile([128, 2], F32, tag="stats2")
                nc.sync.dma_start(stats2[:], stats_out.ap())

                # ---- BN affine params ([128,1] each)
                mu = PS.tile([128, 1], F32, tag="mu")
                nc.vector.tensor_scalar(mu[:], stats2[:, 0:1], 1.0 / N, None,
                                        op0=mybir.AluOpType.mult)
                e2 = PS.tile([128, 1], F32, tag="e2")
                nc.vector.tensor_scalar(e2[:], stats2[:, 1:2], 1.0 / N, None,
                                        op0=mybir.AluOpType.mult)
                var = PS.tile([128, 1], F32, tag="var")
                nc.vector.scalar_tensor_tensor(
                    out=var[:], in0=mu[:], scalar=-1.0, in1=mu[:],
                    op0=mybir.AluOpType.mult, op1=mybir.AluOpType.mult)
                nc.vector.tensor_tensor(out=var[:], in0=e2[:], in1=var[:],
                                        op=mybir.AluOpType.add)
                sd = PS.tile([128, 1], F32, tag="sd")
                nc.scalar.activation(out=sd[:], in_=var[:],
                                     func=mybir.ActivationFunctionType.Sqrt,
                                     bias=eps_sb[:, 0:1])
                rinv = PS.tile([128, 1], F32, tag="rinv")
                nc.vector.reciprocal(rinv[:], sd[:])
                alpha = PS.tile([128, 1], F32, tag="alpha")
                nc.vector.tensor_tensor(out=alpha[:], in0=gam_sb[l][:],
                                        in1=rinv[:], op=mybir.AluOpType.mult)
                bias_p = PS.tile([128, 1], F32, tag="biasp")
                nc.vector.scalar_tensor_tensor(
                    out=bias_p[:], in0=alpha[:], scalar=-1.0, in1=mu[:],
                    op0=mybir.AluOpType.mult, op1=mybir.AluOpType.mult)
                nc.vector.tensor_tensor(out=bias_p[:], in0=bet_sb[l][:],
                                        in1=bias_p[:], op=mybir.AluOpType.add)
                nalpha = PS.tile([128, 1], F32, tag="nalpha")
                nc.vector.tensor_scalar(nalpha[:], alpha[:], -1.0, None,
                                        op0=mybir.AluOpType.mult)
                nbias = PS.tile([128, 1], F32, tag="nbias")
                nc.vector.tensor_scalar(nbias[:], bias_p[:], -1.0, None,
                                        op0=mybir.AluOpType.mult)
                na = PS.tile([128, 1], F32, tag="na")
                nc.vector.tensor_scalar(na[:], a_sb[l][:], -1.0, None,
                                        op0=mybir.AluOpType.mult)

                # ---- fused BN + PReLU: y = relu(z) - a*relu(-z)
                for (o, cw) in chunks:
                    pos = PW.tile([128, 512], F32, tag="pos")
                    nc.scalar.activation(
                        out=pos[:, :cw], in_=actT[:, o : o + cw],
                        func=mybir.ActivationFunctionType.Relu,
                        bias=bias_p[:, :1], scale=alpha[:, :1])
                    neg = PW.tile([128, 512], F32, tag="neg")
                    nc.scalar.activation(
                        out=neg[:, :cw], in_=actT[:, o : o + cw],
                        func=mybir.ActivationFunctionType.Relu,
                        bias=nbias[:, :1], scale=nalpha[:, :1])
                    nc.vector.scalar_tensor_tensor(
                        out=actT[:, o : o + cw], in0=neg[:, :cw],
                        scalar=na[:, :1], in1=pos[:, :cw],
                        op0=mybir.AluOpType.mult, op1=mybir.AluOpType.add)

            # ---- write h2 back as [rows, feat]
            for t in range(NT):
                tp = PT.tile([128, 128], F32, tag="tp")
                nc.tensor.transpose(out=tp[:],
                                    in_=actT[:, 128 * t : 128 * (t + 1)],
                                    identity=ident[:])
                ot = PW.tile([128, 128], F32, tag="ot")
                nc.vector.tensor_copy(ot[:], tp[:])
                nc.sync.dma_start(out_r[t], ot[:])

    nc.compile()
    return nc


# ------------------------------------------------------------------- driver

_CACHE: dict = {}


def _get_compiled(key, N, SH, S, K, band_off, seg_off):
    if key not in _CACHE:
        nc = build_kernel(N, SH, S, K, band_off, seg_off)
        nc.m = get_hw_module(nc.m)
        _CACHE[key] = nc
    return _CACHE[key]


def make_in_maps(pre, w0, b0, gamma0, beta0, a0, w1, b1, gamma1, beta1, a1):
    def col(v):
        return np.ascontiguousarray(np.asarray(v, np.float32).reshape(-1, 1))

    def rep(v):
        return np.full((128, 1), np.float32(np.asarray(v).reshape(-1)[0]),
                       np.float32)

    iota = np.tile(np.arange(PAIR, dtype=np.float16)[None, :], (128, 1))
    maps = []
    for c in range(NB):
        maps.append({
            "x": pre["x_sh"][c],
            "gidx": pre["gidx"][c],
            "dst32": pre["dst32"][c],
            "dinvrow": pre["dinvrow"][c],
            "dinv_cols": pre["dinv_cols"][c],
            "iota256": iota,
            "w0": np.ascontiguousarray(np.asarray(w0, np.float32)),
            "w1": np.ascontiguousarray(np.asarray(w1, np.float32)),
            "gamma0": col(gamma0), "beta0": col(beta0), "a0": rep(a0),
            "gamma1": col(gamma1), "beta1": col(beta1), "a1": rep(a1),
        })
    return maps


def kernel(x, edge_index, w0, b0, gamma0, beta0, a0,
           w1, b1, gamma1, beta1, a1, _trace=False):
    x = np.asarray(x, np.float32)
    edge_index = np.asarray(edge_index, np.int64)
    pre = preprocess(x, edge_index)
    N, nsh, SH, S = pre["N"], pre["nsh"], pre["SH"], pre["S"]
    key = (N, SH, S, pre["K"], pre["band_off"])
    nc = _get_compiled(key, N, SH, S, pre["K"], pre["band_off"],
                       pre["seg_off"])
    in_maps = make_in_maps(pre, w0, b0, gamma0, beta0, a0,
                           w1, b1, gamma1, beta1, a1)
    res = bass_utils.run_bass_kernel_spmd(
        nc, in_maps, core_ids=list(range(NB)), trace=_trace)
    out = np.concatenate([res.results[c]["out"][:nsh] for c in range(NB)],
                         axis=0)[:N]
    if _trace:
        kernel.last_results = res
    return np.ascontiguousarray(out)


# revision 9
# speedup vs baseline: 1.1887x; 1.1887x over previous
"""GCN encoder (2x GCNConv + BatchNorm + PReLU) on 8 Trainium2 NeuronCores.

Full inputs in, full outputs out. v2 design:
  - nodes sharded contiguously across 8 cores (12500 real rows + pad -> 12544),
  - v' = dinv_src * (h @ W) computed locally per core, cast to fp16,
  - AllGather of the fp16 v' table (halo = everything on a random graph),
  - per-edge messages fetched with gpsimd.dma_gather across 4 SWDGE queues
    (descriptor prep parallelizes across queues; this was the v1 bottleneck),
  - segment-sum done ON THE TENSOR ENGINE: edges are sorted by dst tile-pair,
    each 128-edge chunk is multiplied by a DVE-built one-hot selector
    [128 edges x 256 dst slots] and accumulated into a PSUM tile per
    dst-pair -- no dma_scatter_add, no HBM accumulator round trip,
  - conv = dinv_dst * (psum + dinv_dst * v) adds the self-loop analytically,
  - BN stats via free-dim reduce in [feat, rows] layout, stats AllReduce,
    fused BN+PReLU via two ACT Relu passes + one DVE scalar_tensor_tensor.

Stream layout (identical structure on all 8 cores -- SPMD): edges sorted by
(slab, dst-pair) where slab = vfull_row // 32768 (dma_gather idxs are int16,
so each call reads one <=32768-row slab of the AllGathered table). Each
(slab, pair) segment is padded to a 128 multiple with pointers to a known
all-zero table row, and segment sizes are maxed over cores so the compiled
chunk->pair structure is core-independent. Bands (slabs) are padded to the
2048-idx call size, so call k always covers stream chunks [16k, 16k+16).
"""

import numpy as np

import concourse.bass as bass
import concourse.bacc as bacc
import concourse.tile as tile
from concourse import mybir
from concourse import bass_utils
from concourse.masks import make_identity
from concourse.bass_interp import get_hw_module

F32 = mybir.dt.float32
F16 = mybir.dt.float16
I16 = mybir.dt.int16
EPS = 1e-5
NB = 8          # cores
D = 128
SLAB = 32768    # int16 index range per gather call
CALLSZ = 4096   # max idxs per dma_gather call (32 chunks)
PAIR = 256      # dst slots per psum accumulation tile


# ---------------------------------------------------------------- host side


def preprocess(x: np.ndarray, edge_index: np.ndarray):
    N = x.shape[0]
    nsh = (N + NB - 1) // NB                     # 12500 real rows per shard
    SH = ((nsh + 1 + 127) // 128) * 128          # 12544 padded rows
    NT = SH // 128                               # 98 tiles
    NP = (SH + PAIR - 1) // PAIR                 # 49 dst pairs
    NSLAB = (NB * SH + SLAB - 1) // SLAB         # 4 slabs over vfull

    src = edge_index[0]
    dst = edge_index[1]
    deg = np.bincount(dst, minlength=N) + 1      # +1: self-loop
    dinv = (1.0 / np.sqrt(deg.astype(np.float64))).astype(np.float32)

    # relative index of a guaranteed all-zero vloc row inside each slab
    zrel = []
    for j in range(NSLAB):
        base = j * SLAB
        zr = None
        for b in range(NB):
            z0 = b * SH + nsh                    # first zero row of block b
            if base <= z0 < min(base + SLAB, NB * SH):
                zr = z0 - base
                break
        assert zr is not None
        zrel.append(zr)

    # per-core edge lists sorted by (slab, dst pair)
    per_core = []
    counts = np.zeros((NB, NSLAB, NP), np.int64)
    for c in range(NB):
        m = (dst // nsh) == c
        gs = src[m]
        sl = (dst[m] - c * nsh).astype(np.int64)
        vrow = (gs // nsh) * SH + (gs % nsh)
        slab = vrow // SLAB
        P = sl // PAIR
        order = np.lexsort((P, slab))
        vrow, sl, slab, P = vrow[order], sl[order], slab[order], P[order]
        per_core.append((vrow, sl, slab, P))
        counts[c] = np.bincount(slab * NP + P,
                                minlength=NSLAB * NP).reshape(NSLAB, NP)

    K = np.ceil(counts.max(axis=0) / 128).astype(np.int64)   # [NSLAB, NP]
    seg_rows = K * 128

    # band-major stream layout; each band padded to a CALLSZ multiple
    band_rows = seg_rows.sum(axis=1)
    band_cap = ((band_rows + 127) // 128) * 128
    band_cap = np.maximum(band_cap, 128)
    band_off = np.concatenate([[0], np.cumsum(band_cap)])
    S = int(band_off[-1])                        # total stream rows
    seg_off = np.zeros((NSLAB, NP), np.int64)
    for j in range(NSLAB):
        seg_off[j] = band_off[j] + np.concatenate(
            [[0], np.cumsum(seg_rows[j])[:-1]])

    # fill per-core index + dst tables
    gidx = np.zeros((NB, 128, S // 16), np.int16)
    dst16 = np.zeros((NB, 128, S // 128), np.float16)
    dinvrow = np.zeros((NB, 128, SH), np.float16)
    dinv_cols = np.zeros((NB, 128, NT), np.float32)
    x_sh = np.zeros((NB, SH, D), np.float32)
    for c in range(NB):
        g = np.zeros(S, np.int64)
        for j in range(NSLAB):
            g[band_off[j] : band_off[j + 1]] = zrel[j]
        dv = np.full(S, 300.0, np.float64)
        vrow, sl, slab, P = per_core[c]
        # slot position for each edge: segment start + rank within segment
        segid = slab * NP + P
        seg_start = seg_off.reshape(-1)[segid]
        # edges are sorted by segid, so rank = index - first index of segid
        first = np.searchsorted(segid, segid, side="left")
        pos = seg_start + (np.arange(len(segid)) - first)
        g[pos] = vrow - slab * SLAB
        dv[pos] = sl % PAIR
        gidx[c] = np.tile(
            np.ascontiguousarray(g.astype(np.int16).reshape(S // 16, 16).T),
            (8, 1))
        dst16[c] = np.ascontiguousarray(
            dv.astype(np.float16).reshape(S // 128, 128).T)

        lo = c * nsh
        hi = min(lo + nsh, N)
        dloc = np.zeros(SH, np.float32)
        dloc[: hi - lo] = dinv[lo:hi]
        dinvrow[c] = np.tile(dloc.astype(np.float16)[None, :], (128, 1))
        dinv_cols[c] = dloc.reshape(NT, 128).T
        x_sh[c, : hi - lo] = x[lo:hi]

    return dict(
        N=N, nsh=nsh, SH=SH, S=S,
        K=tuple(map(tuple, K)), band_off=tuple(int(b) for b in band_off),
        seg_off=tuple(map(tuple, seg_off)),
        gidx=gidx, dst16=dst16, dinvrow=dinvrow, dinv_cols=dinv_cols,
        x_sh=x_sh,
    )


# -------------------------------------------------------------- device side


def build_kernel(N: int, SH: int, S: int, K, band_off, seg_off):
    nc = bacc.Bacc("TRN2", target_bir_lowering=False, debug=False,
                   num_devices=NB, num_swdge_queues=4)
    rg = [list(range(NB))]
    NT = SH // 128
    NP = SH // PAIR
    NSLAB = len(band_off) - 1
    chunks = [(o, min(512, SH - o)) for o in range(0, SH, 512)]

    x_in = nc.dram_tensor("x", [SH, D], F32, kind="ExternalInput")
    gidx_in = nc.dram_tensor("gidx", [128, S // 16], I16,
                             kind="ExternalInput")
    dst_in = nc.dram_tensor("dst16", [128, S // 128], F16,
                            kind="ExternalInput")
    dnr_in = nc.dram_tensor("dinvrow", [128, SH], F16, kind="ExternalInput")
    dinv_in = nc.dram_tensor("dinv_cols", [128, NT], F32,
                             kind="ExternalInput")
    iota_in = nc.dram_tensor("iota256", [128, PAIR], F16,
                             kind="ExternalInput")
    w_in = [nc.dram_tensor(f"w{l}", [D, D], F32, kind="ExternalInput")
            for l in range(2)]
    gam_in = [nc.dram_tensor(f"gamma{l}", [D, 1], F32, kind="ExternalInput")
              for l in range(2)]
    bet_in = [nc.dram_tensor(f"beta{l}", [D, 1], F32, kind="ExternalInput")
              for l in range(2)]
    a_in = [nc.dram_tensor(f"a{l}", [D, 1], F32, kind="ExternalInput")
            for l in range(2)]
    out_t = nc.dram_tensor("out", [SH, D], F32, kind="ExternalOutput")

    vloc = nc.dram_tensor("vloc", [SH, D], F16)
    vfull = nc.dram_tensor("vfull", [NB * SH, D], F16, addr_space="Shared")
    stats_in = nc.dram_tensor("stats_in", [D, 2], F32)
    stats_out = nc.dram_tensor("stats_out", [D, 2], F32, addr_space="Shared")

    out_r = out_t.ap().rearrange("(t p) f -> t p f", p=128)
    x_r = x_in.ap().rearrange("(t p) f -> t p f", p=128)
    vloc_r = vloc.ap().rearrange("(t p) f -> t p f", p=128)


    with tile.TileContext(nc) as tc:
        with (
            tc.tile_pool(name="pers", bufs=1) as PE_,
            tc.tile_pool(name="act", bufs=1) as PA_,
            tc.tile_pool(name="msg", bufs=7) as PM,
            tc.tile_pool(name="sel", bufs=4) as PSL,
            tc.tile_pool(name="work", bufs=3) as PW,
            tc.tile_pool(name="small", bufs=2) as PS,
            tc.tile_pool(name="psP", bufs=3, space="PSUM") as PP,
            tc.tile_pool(name="psV", bufs=1, space="PSUM") as PV,
            tc.tile_pool(name="psA", bufs=1, space="PSUM") as PAP,
            tc.tile_pool(name="psT", bufs=1, space="PSUM") as PT,
        ):
            ident = PE_.tile([128, 128], F32, tag="ident")
            make_identity(nc, ident[:])
            ident16 = PE_.tile([128, 128], F16, tag="ident16")
            nc.vector.tensor_copy(ident16[:], ident[:])
            gidx_sb = PE_.tile([128, S // 16], I16, tag="gidx")
            nc.sync.dma_start(gidx_sb[:], gidx_in.ap())
            dst_sb = PE_.tile([128, S // 128], F16, tag="dst16")
            nc.sync.dma_start(dst_sb[:], dst_in.ap())
            dnr_sb = PE_.tile([128, SH], F16, tag="dinvrow")
            nc.sync.dma_start(dnr_sb[:], dnr_in.ap())
            dinv_sb = PE_.tile([128, NT], F32, tag="dinv")
            nc.sync.dma_start(dinv_sb[:], dinv_in.ap())
            iota_sb = PE_.tile([128, PAIR], F16, tag="iota")
            nc.sync.dma_start(iota_sb[:], iota_in.ap())
            w_sb, gam_sb, bet_sb, a_sb = [], [], [], []
            for l in range(2):
                w_sb.append(PE_.tile([128, 128], F32, tag=f"w{l}",
                                     name=f"w{l}_sb"))
                nc.sync.dma_start(w_sb[l][:], w_in[l].ap())
                gam_sb.append(PE_.tile([128, 1], F32, tag=f"g{l}", name=f"g{l}_sb"))
                nc.sync.dma_start(gam_sb[l][:], gam_in[l].ap())
                bet_sb.append(PE_.tile([128, 1], F32, tag=f"b{l}", name=f"b{l}_sb"))
                nc.sync.dma_start(bet_sb[l][:], bet_in[l].ap())
                a_sb.append(PE_.tile([128, 1], F32, tag=f"a{l}", name=f"a{l}_sb"))
                nc.sync.dma_start(a_sb[l][:], a_in[l].ap())
            zero_sb = PE_.tile([128, 128], F32, tag="zero")
            nc.vector.memset(zero_sb[:], 0.0)
            eps_sb = PE_.tile([128, 1], F32, tag="eps")
            nc.vector.memset(eps_sb[:], EPS)

            actT = PA_.tile([128, SH], F32, tag="actT")  # h as [feat, rows]

            # ---- load x, transpose into actT
            for t in range(NT):
                xt = PW.tile([128, 128], F32, tag="xt")
                nc.sync.dma_start(xt[:], x_r[t])
                tp = PT.tile([128, 128], F32, tag="tp")
                nc.tensor.transpose(out=tp[:], in_=xt[:], identity=ident[:])
                nc.vector.tensor_copy(actT[:, 128 * t : 128 * (t + 1)], tp[:])

            for l in range(2):
                # ---- v' table: vloc[t] = f16(dinv_src * (W.T @ actT)[.,t].T)
                for (o, cw) in chunks:
                    vp = PAP.tile([128, 512], F32, tag="vp")
                    nc.tensor.matmul(out=vp[:, :cw], lhsT=w_sb[l][:],
                                     rhs=actT[:, o : o + cw],
                                     start=True, stop=True)
                    vt = PW.tile([128, 512], F16, tag="vt")
                    nc.vector.tensor_copy(vt[:, :cw], vp[:, :cw])
                    for s in range(0, cw, 128):
                        t = (o + s) // 128
                        tp = PT.tile([128, 128], F16, tag="tph")
                        nc.tensor.transpose(out=tp[:], in_=vt[:, s : s + 128],
                                            identity=ident16[:])
                        vv = PW.tile([128, 128], F16, tag="vv")
                        nc.vector.tensor_scalar(
                            vv[:], tp[:], dinv_sb[:, t : t + 1], None,
                            op0=mybir.AluOpType.mult)
                        nc.sync.dma_start(vloc_r[t], vv[:])

                # ---- halo exchange: AllGather the fp16 table
                nc.gpsimd.collective_compute(
                    "AllGather", mybir.AluOpType.bypass, replica_groups=rg,
                    ins=[vloc.ap().opt()], outs=[vfull.ap().opt()])

                # ---- gather + selector-matmul scatter, pair by pair
                call_tiles = [None] * (S // 128)
                band_next = [band_off[j] for j in range(NSLAB)]

                def issue_through(P):
                    # gather everything needed for pairs <= P, per band
                    for j in range(NSLAB):
                        limit = seg_off[j][P] + 128 * K[j][P]
                        while (band_next[j] < band_off[j + 1]
                               and band_next[j] < limit):
                            pos = band_next[j]
                            sz = min(CALLSZ, band_off[j + 1] - pos)
                            k = pos // 128          # chunk index of call
                            mt = PM.tile([128, CALLSZ // 128, 128], F16,
                                         tag="mt", name=f"mt{l}_{k}")
                            nc.gpsimd.dma_gather(
                                out_ap=mt[:, : sz // 128, :],
                                in_ap=vfull.ap()[j * SLAB :
                                                 min((j + 1) * SLAB,
                                                     NB * SH), :],
                                idxs_ap=gidx_sb[:, pos // 16 :
                                                (pos + sz) // 16],
                                num_idxs=sz, num_idxs_reg=sz,
                                elem_size=D, single_packet=False,
                                queue_num=qn[0] % 4)
                            qn[0] += 1
                            for cc in range(sz // 128):
                                call_tiles[k + cc] = (mt, cc)
                            band_next[j] += sz

                qn = [0]
                for P in range(NP):
                    issue_through(min(P + 2, NP - 1))
                    pcw = min(PAIR, SH - P * PAIR)
                    ps = PP.tile([128, PAIR], F32, tag="ps")
                    # chunk list for this pair across bands
                    pchunks = []
                    for j in range(NSLAB):
                        g0 = seg_off[j][P] // 128
                        pchunks.append((g0, K[j][P]))
                    total = sum(k for _, k in pchunks)
                    done = 0
                    for (g0, kk) in pchunks:
                        if kk == 0:
                            continue
                        sel = PSL.tile([128, kk, PAIR], F16, tag=f"sel{kk}",
                                       name=f"sel{P}_{kk}")
                        nc.vector.tensor_tensor(
                            out=sel[:],
                            in0=iota_sb[:].unsqueeze(1).broadcast_to(
                                [128, kk, PAIR]),
                            in1=dst_sb[:, g0 : g0 + kk].unsqueeze(2)
                                .broadcast_to([128, kk, PAIR]),
                            op=mybir.AluOpType.is_equal)
                        for i in range(kk):
                            g = g0 + i
                            mt, cc = call_tiles[g]
                            nc.tensor.matmul(
                                out=ps[:, :pcw],
                                lhsT=mt[:, cc, :],
                                rhs=sel[:, i, :pcw],
                                start=(done == 0), stop=(done == total - 1))
                            done += 1
                    # ---- conv = dinv*(ps + dinv*v); v recomputed on PE
                    vq = PV.tile([128, PAIR], F32, tag="vq")
                    nc.tensor.matmul(out=vq[:, :pcw], lhsT=w_sb[l][:],
                                     rhs=actT[:, P * PAIR : P * PAIR + pcw],
                                     start=True, stop=True)
                    dsl = dnr_sb[:, P * PAIR : P * PAIR + pcw]
                    t1 = PW.tile([128, PAIR], F32, tag="t1")
                    nc.vector.tensor_tensor(out=t1[:, :pcw], in0=vq[:, :pcw],
                                            in1=dsl, op=mybir.AluOpType.mult)
                    t2 = PW.tile([128, PAIR], F32, tag="t2")
                    nc.vector.tensor_tensor(out=t2[:, :pcw], in0=ps[:, :pcw],
                                            in1=t1[:, :pcw],
                                            op=mybir.AluOpType.add)
                    nc.vector.tensor_tensor(
                        out=actT[:, P * PAIR : P * PAIR + pcw],
                        in0=t2[:, :pcw], in1=dsl, op=mybir.AluOpType.mult)

                # ---- BN stats (biased, over the real N rows; pad rows are 0)
                nk = len(chunks)
                sumc = PS.tile([128, nk], F32, tag="sumc")
                sqc = PS.tile([128, nk], F32, tag="sqc")
                for k, (o, cw) in enumerate(chunks):
                    nc.vector.tensor_reduce(
                        out=sumc[:, k : k + 1], in_=actT[:, o : o + cw],
                        axis=mybir.AxisListType.X, op=mybir.AluOpType.add)
                    sq = PW.tile([128, 512], F32, tag="sq")
                    nc.scalar.activation(
                        out=sq[:, :cw], in_=actT[:, o : o + cw],
                        func=mybir.ActivationFunctionType.Square,
                        bias=zero_sb[:, 0:1],
                        accum_out=sqc[:, k : k + 1])
                stats_sb = PS.tile([128, 2], F32, tag="stats")
                nc.vector.tensor_reduce(out=stats_sb[:, 0:1], in_=sumc[:],
                                        axis=mybir.AxisListType.X,
                                        op=mybir.AluOpType.add)
                nc.vector.tensor_reduce(out=stats_sb[:, 1:2], in_=sqc[:],
                                        axis=mybir.AxisListType.X,
                                        op=mybir.AluOpType.add)
                nc.sync.dma_start(stats_in.ap(), stats_sb[:])
                nc.gpsimd.collective_compute(
                    "AllReduce", mybir.AluOpType.add, replica_groups=rg,
                    ins=[stats_in.ap().opt()], outs=[stats_out.ap().opt()])
                stats2 = PS.tile([128, 2], F32, tag="stats2")
                nc.sync.dma_start(stats2[:], stats_out.ap())

                # ---- BN affine params ([128,1] each)
                mu = PS.tile([128, 1], F32, tag="mu")
                nc.vector.tensor_scalar(mu[:], stats2[:, 0:1], 1.0 / N, None,
                                        op0=mybir.AluOpType.mult)
                e2 = PS.tile([128, 1], F32, tag="e2")
                nc.vector.tensor_scalar(e2[:], stats2[:, 1:2], 1.0 / N, None,
                                        op0=mybir.AluOpType.mult)
                var = PS.tile([128, 1], F32, tag="var")
                nc.vector.scalar_tensor_tensor(
                    out=var[:], in0=mu[:], scalar=-1.0, in1=mu[:],
                    op0=mybir.AluOpType.mult, op1=mybir.AluOpType.mult)
                nc.vector.tensor_tensor(out=var[:], in0=e2[:], in1=var[:],
                                        op=mybir.AluOpType.add)
                sd = PS.tile([128, 1], F32, tag="sd")
                nc.scalar.activation(out=sd[:], in_=var[:],
                                     func=mybir.ActivationFunctionType.Sqrt,
                                     bias=eps_sb[:, 0:1])
                rinv = PS.tile([128, 1], F32, tag="rinv")
                nc.vector.reciprocal(rinv[:], sd[:])
                alpha = PS.tile([128, 1], F32, tag="alpha")
                nc.vector.tensor_tensor(out=alpha[:], in0=gam_sb[l][:],
                                        in1=rinv[:], op=mybir.AluOpType.mult)
                bias_p = PS.tile([128, 1], F32, tag="biasp")
                nc.vector.scalar_tensor_tensor(
                    out=bias_p[:], in0=alpha[:], scalar=-1.0, in1=mu[:],
                    op0=mybir.AluOpType.mult, op1=mybir.AluOpType.mult)
                nc.vector.tensor_tensor(out=bias_p[:], in0=bet_sb[l][:],
                                        in1=bias_p[:], op=mybir.AluOpType.add)
                nalpha = PS.tile([128, 1], F32, tag="nalpha")
                nc.vector.tensor_scalar(nalpha[:], alpha[:], -1.0, None,
                                        op0=mybir.AluOpType.mult)
                nbias = PS.tile([128, 1], F32, tag="nbias")
                nc.vector.tensor_scalar(nbias[:], bias_p[:], -1.0, None,
                                        op0=mybir.AluOpType.mult)
                na = PS.tile([128, 1], F32, tag="na")
                nc.vector.tensor_scalar(na[:], a_sb[l][:], -1.0, None,
                                        op0=mybir.AluOpType.mult)

                # ---- fused BN + PReLU: y = relu(z) - a*relu(-z)
                for (o, cw) in chunks:
                    pos = PW.tile([128, 512], F32, tag="pos")
                    nc.scalar.activation(
                        out=pos[:, :cw], in_=actT[:, o : o + cw],
                        func=mybir.ActivationFunctionType.Relu,
                        bias=bias_p[:, :1], scale=alpha[:, :1])
                    neg = PW.tile([128, 512], F32, tag="neg")
                    nc.scalar.activation(
                        out=neg[:, :cw], in_=actT[:, o : o + cw],
                        func=mybir.ActivationFunctionType.Relu,
                        bias=nbias[:, :1], scale=nalpha[:, :1])
                    nc.vector.scalar_tensor_tensor(
                        out=actT[:, o : o + cw], in0=neg[:, :cw],
                        scalar=na[:, :1], in1=pos[:, :cw],
                        op0=mybir.AluOpType.mult, op1=mybir.AluOpType.add)

            # ---- write h2 back as [rows, feat]
            for t in range(NT):
                tp = PT.tile([128, 128], F32, tag="tp")
                nc.tensor.transpose(out=tp[:],
                                    in_=actT[:, 128 * t : 128 * (t + 1)],
                                    identity=ident[:])
                ot = PW.tile([128, 128], F32, tag="ot")
                nc.vector.tensor_copy(ot[:], tp[:])
                nc.sync.dma_start(out_r[t], ot[:])

    nc.compile()
    return nc


# ------------------------------------------------------------------- driver

_CACHE: dict = {}


def _get_compiled(key, N, SH, S, K, band_off, seg_off):
    if key not in _CACHE:
        nc = build_kernel(N, SH, S, K, band_off, seg_off)
        nc.m = get_hw_module(nc.m)
        _CACHE[key] = nc
    return _CACHE[key]


def make_in_maps(pre, w0, b0, gamma0, beta0, a0, w1, b1, gamma1, beta1, a1):
    def col(v):
        return np.ascontiguousarray(np.asarray(v, np.float32).reshape(-1, 1))

    def rep(v):
        return np.full((128, 1), np.float32(np.asarray(v).reshape(-1)[0]),
                       np.float32)

    iota = np.tile(np.arange(PAIR, dtype=np.float16)[None, :], (128, 1))
    maps = []
    for c in range(NB):
        maps.append({
            "x": pre["x_sh"][c],
            "gidx": pre["gidx"][c],
            "dst16": pre["dst16"][c],
            "dinvrow": pre["dinvrow"][c],
            "dinv_cols": pre["dinv_cols"][c],
            "iota256": iota,
            "w0": np.ascontiguousarray(np.asarray(w0, np.float32)),
            "w1": np.ascontiguousarray(np.asarray(w1, np.float32)),
            "gamma0": col(gamma0), "beta0": col(beta0), "a0": rep(a0),
            "gamma1": col(gamma1), "beta1": col(beta1), "a1": rep(a1),
        })
    return maps


def kernel(x, edge_index, w0, b0, gamma0, beta0, a0,
           w1, b1, gamma1, beta1, a1, _trace=False):
    x = np.asarray(x, np.float32)
    edge_index = np.asarray(edge_index, np.int64)
    pre = preprocess(x, edge_index)
    N, nsh, SH, S = pre["N"], pre["nsh"], pre["SH"], pre["S"]
    key = (N, SH, S, pre["K"], pre["band_off"])
    nc = _get_compiled(key, N, SH, S, pre["K"], pre["band_off"],
                       pre["seg_off"])
    in_maps = make_in_maps(pre, w0, b0, gamma0, beta0, a0,
                           w1, b1, gamma1, beta1, a1)
    res = bass_utils.run_bass_kernel_spmd(
        nc, in_maps, core_ids=list(range(NB)), trace=_trace)
    out = np.concatenate([res.results[c]["out"][:nsh] for c in range(NB)],
                         axis=0)[:N]
    if _trace:
        kernel.last_results = res
    return np.ascontiguousarray(out)


# revision 10
# speedup vs baseline: 1.4982x; 1.2604x over previous
"""GCN encoder (2x GCNConv + BatchNorm + PReLU) on 8 Trainium2 NeuronCores.

Full inputs in, full outputs out. v2 design:
  - nodes sharded contiguously across 8 cores (12500 real rows + pad -> 12544),
  - v' = dinv_src * (h @ W) computed locally per core, cast to fp16,
  - AllGather of the fp16 v' table (halo = everything on a random graph),
  - per-edge messages fetched with gpsimd.dma_gather across 4 SWDGE queues
    (descriptor prep parallelizes across queues; this was the v1 bottleneck),
  - segment-sum done ON THE TENSOR ENGINE: edges are sorted by dst tile-pair,
    each 128-edge chunk is multiplied by a DVE-built one-hot selector
    [128 edges x 256 dst slots] and accumulated into a PSUM tile per
    dst-pair -- no dma_scatter_add, no HBM accumulator round trip,
  - conv = dinv_dst * (psum + dinv_dst * v) adds the self-loop analytically,
  - BN stats via free-dim reduce in [feat, rows] layout, stats AllReduce,
    fused BN+PReLU via two ACT Relu passes + one DVE scalar_tensor_tensor.

Stream layout (identical structure on all 8 cores -- SPMD): edges sorted by
(slab, dst-pair) where slab = vfull_row // 32768 (dma_gather idxs are int16,
so each call reads one <=32768-row slab of the AllGathered table). Each
(slab, pair) segment is padded to a 128 multiple with pointers to a known
all-zero table row, and segment sizes are maxed over cores so the compiled
chunk->pair structure is core-independent. Bands (slabs) are padded to the
2048-idx call size, so call k always covers stream chunks [16k, 16k+16).
"""

import numpy as np

import concourse.bass as bass
import concourse.bacc as bacc
import concourse.tile as tile
from concourse import mybir
from concourse import bass_utils
from concourse.masks import make_identity
from concourse.bass_interp import get_hw_module

F32 = mybir.dt.float32
F16 = mybir.dt.float16
I16 = mybir.dt.int16
EPS = 1e-5
NB = 8          # cores
D = 128
SLAB = 32768    # int16 index range per gather call
CALLSZ = 2048   # max idxs per dma_gather call (16 chunks)
PAIR = 256      # dst slots per psum accumulation tile


# ---------------------------------------------------------------- host side


def preprocess(x: np.ndarray, edge_index: np.ndarray):
    N = x.shape[0]
    nsh = (N + NB - 1) // NB                     # 12500 real rows per shard
    SH = ((nsh + 1 + 127) // 128) * 128          # 12544 padded rows
    NT = SH // 128                               # 98 tiles
    NP = (SH + PAIR - 1) // PAIR                 # 49 dst pairs
    NSLAB = (NB * SH + SLAB - 1) // SLAB         # 4 slabs over vfull

    src = edge_index[0]
    dst = edge_index[1]
    deg = np.bincount(dst, minlength=N) + 1      # +1: self-loop
    dinv = (1.0 / np.sqrt(deg.astype(np.float64))).astype(np.float32)

    # relative index of a guaranteed all-zero vloc row inside each slab
    zrel = []
    for j in range(NSLAB):
        base = j * SLAB
        zr = None
        for b in range(NB):
            z0 = b * SH + nsh                    # first zero row of block b
            if base <= z0 < min(base + SLAB, NB * SH):
                zr = z0 - base
                break
        assert zr is not None
        zrel.append(zr)

    # per-core edge lists sorted by (slab, dst pair)
    per_core = []
    counts = np.zeros((NB, NSLAB, NP), np.int64)
    for c in range(NB):
        m = (dst // nsh) == c
        gs = src[m]
        sl = (dst[m] - c * nsh).astype(np.int64)
        vrow = (gs // nsh) * SH + (gs % nsh)
        slab = vrow // SLAB
        P = sl // PAIR
        order = np.lexsort((P, slab))
        vrow, sl, slab, P = vrow[order], sl[order], slab[order], P[order]
        per_core.append((vrow, sl, slab, P))
        counts[c] = np.bincount(slab * NP + P,
                                minlength=NSLAB * NP).reshape(NSLAB, NP)

    K = np.ceil(counts.max(axis=0) / 128).astype(np.int64)   # [NSLAB, NP]
    seg_rows = K * 128

    # band-major stream layout; each band padded to a CALLSZ multiple
    band_rows = seg_rows.sum(axis=1)
    band_cap = ((band_rows + 127) // 128) * 128
    band_cap = np.maximum(band_cap, 128)
    band_off = np.concatenate([[0], np.cumsum(band_cap)])
    S = int(band_off[-1])                        # total stream rows
    seg_off = np.zeros((NSLAB, NP), np.int64)
    for j in range(NSLAB):
        seg_off[j] = band_off[j] + np.concatenate(
            [[0], np.cumsum(seg_rows[j])[:-1]])

    # fill per-core index + dst tables
    gidx = np.zeros((NB, 128, S // 16), np.int16)
    dst16 = np.zeros((NB, 128, S // 128), np.float16)
    dinvrow = np.zeros((NB, 128, SH), np.float16)
    dinv_cols = np.zeros((NB, 128, NT), np.float32)
    x_sh = np.zeros((NB, SH, D), np.float32)
    for c in range(NB):
        g = np.zeros(S, np.int64)
        for j in range(NSLAB):
            g[band_off[j] : band_off[j + 1]] = zrel[j]
        dv = np.full(S, 300.0, np.float64)
        vrow, sl, slab, P = per_core[c]
        # slot position for each edge: segment start + rank within segment
        segid = slab * NP + P
        seg_start = seg_off.reshape(-1)[segid]
        # edges are sorted by segid, so rank = index - first index of segid
        first = np.searchsorted(segid, segid, side="left")
        pos = seg_start + (np.arange(len(segid)) - first)
        g[pos] = vrow - slab * SLAB
        dv[pos] = sl % PAIR
        gidx[c] = np.tile(
            np.ascontiguousarray(g.astype(np.int16).reshape(S // 16, 16).T),
            (8, 1))
        dst16[c] = np.ascontiguousarray(
            dv.astype(np.float16).reshape(S // 128, 128).T)

        lo = c * nsh
        hi = min(lo + nsh, N)
        dloc = np.zeros(SH, np.float32)
        dloc[: hi - lo] = dinv[lo:hi]
        dinvrow[c] = np.tile(dloc.astype(np.float16)[None, :], (128, 1))
        dinv_cols[c] = dloc.reshape(NT, 128).T
        x_sh[c, : hi - lo] = x[lo:hi]

    return dict(
        N=N, nsh=nsh, SH=SH, S=S,
        K=tuple(map(tuple, K)), band_off=tuple(int(b) for b in band_off),
        seg_off=tuple(map(tuple, seg_off)),
        gidx=gidx, dst16=dst16, dinvrow=dinvrow, dinv_cols=dinv_cols,
        x_sh=x_sh,
    )


# -------------------------------------------------------------- device side


def build_kernel(N: int, SH: int, S: int, K, band_off, seg_off):
    nc = bacc.Bacc("TRN2", target_bir_lowering=False, debug=False,
                   num_devices=NB, num_swdge_queues=4)
    rg = [list(range(NB))]
    NT = SH // 128
    NP = SH // PAIR
    NSLAB = len(band_off) - 1
    chunks = [(o, min(512, SH - o)) for o in range(0, SH, 512)]

    x_in = nc.dram_tensor("x", [SH, D], F32, kind="ExternalInput")
    gidx_in = nc.dram_tensor("gidx", [128, S // 16], I16,
                             kind="ExternalInput")
    dst_in = nc.dram_tensor("dst16", [128, S // 128], F16,
                            kind="ExternalInput")
    dnr_in = nc.dram_tensor("dinvrow", [128, SH], F16, kind="ExternalInput")
    dinv_in = nc.dram_tensor("dinv_cols", [128, NT], F32,
                             kind="ExternalInput")
    iota_in = nc.dram_tensor("iota256", [128, PAIR], F16,
                             kind="ExternalInput")
    w_in = [nc.dram_tensor(f"w{l}", [D, D], F32, kind="ExternalInput")
            for l in range(2)]
    gam_in = [nc.dram_tensor(f"gamma{l}", [D, 1], F32, kind="ExternalInput")
              for l in range(2)]
    bet_in = [nc.dram_tensor(f"beta{l}", [D, 1], F32, kind="ExternalInput")
              for l in range(2)]
    a_in = [nc.dram_tensor(f"a{l}", [D, 1], F32, kind="ExternalInput")
            for l in range(2)]
    out_t = nc.dram_tensor("out", [SH, D], F32, kind="ExternalOutput")

    vloc = nc.dram_tensor("vloc", [SH, D], F16)
    vfull = nc.dram_tensor("vfull", [NB * SH, D], F16, addr_space="Shared")
    stats_in = nc.dram_tensor("stats_in", [D, 2], F32)
    stats_out = nc.dram_tensor("stats_out", [D, 2], F32, addr_space="Shared")

    out_r = out_t.ap().rearrange("(t p) f -> t p f", p=128)
    x_r = x_in.ap().rearrange("(t p) f -> t p f", p=128)
    vloc_r = vloc.ap().rearrange("(t p) f -> t p f", p=128)


    with tile.TileContext(nc) as tc:
        with (
            tc.tile_pool(name="pers", bufs=1) as PE_,
            tc.tile_pool(name="act", bufs=1) as PA_,
            tc.tile_pool(name="msg", bufs=12) as PM,
            tc.tile_pool(name="sel", bufs=4) as PSL,
            tc.tile_pool(name="work", bufs=3) as PW,
            tc.tile_pool(name="small", bufs=2) as PS,
            tc.tile_pool(name="psP", bufs=3, space="PSUM") as PP,
            tc.tile_pool(name="psV", bufs=1, space="PSUM") as PV,
            tc.tile_pool(name="psA", bufs=1, space="PSUM") as PAP,
            tc.tile_pool(name="psT", bufs=1, space="PSUM") as PT,
        ):
            ident = PE_.tile([128, 128], F32, tag="ident")
            make_identity(nc, ident[:])
            ident16 = PE_.tile([128, 128], F16, tag="ident16")
            nc.vector.tensor_copy(ident16[:], ident[:])
            gidx_sb = PE_.tile([128, S // 16], I16, tag="gidx")
            nc.sync.dma_start(gidx_sb[:], gidx_in.ap())
            dst_sb = PE_.tile([128, S // 128], F16, tag="dst16")
            nc.sync.dma_start(dst_sb[:], dst_in.ap())
            dnr_sb = PE_.tile([128, SH], F16, tag="dinvrow")
            nc.sync.dma_start(dnr_sb[:], dnr_in.ap())
            dinv_sb = PE_.tile([128, NT], F32, tag="dinv")
            nc.sync.dma_start(dinv_sb[:], dinv_in.ap())
            iota_sb = PE_.tile([128, PAIR], F16, tag="iota")
            nc.sync.dma_start(iota_sb[:], iota_in.ap())
            w_sb, gam_sb, bet_sb, a_sb = [], [], [], []
            for l in range(2):
                w_sb.append(PE_.tile([128, 128], F32, tag=f"w{l}",
                                     name=f"w{l}_sb"))
                nc.sync.dma_start(w_sb[l][:], w_in[l].ap())
                gam_sb.append(PE_.tile([128, 1], F32, tag=f"g{l}", name=f"g{l}_sb"))
                nc.sync.dma_start(gam_sb[l][:], gam_in[l].ap())
                bet_sb.append(PE_.tile([128, 1], F32, tag=f"b{l}", name=f"b{l}_sb"))
                nc.sync.dma_start(bet_sb[l][:], bet_in[l].ap())
                a_sb.append(PE_.tile([128, 1], F32, tag=f"a{l}", name=f"a{l}_sb"))
                nc.sync.dma_start(a_sb[l][:], a_in[l].ap())
            zero_sb = PE_.tile([128, 128], F32, tag="zero")
            nc.vector.memset(zero_sb[:], 0.0)
            eps_sb = PE_.tile([128, 1], F32, tag="eps")
            nc.vector.memset(eps_sb[:], EPS)

            actT = PA_.tile([128, SH], F32, tag="actT")  # h as [feat, rows]

            # ---- load x, transpose into actT
            for t in range(NT):
                xt = PW.tile([128, 128], F32, tag="xt")
                nc.sync.dma_start(xt[:], x_r[t])
                tp = PT.tile([128, 128], F32, tag="tp")
                nc.tensor.transpose(out=tp[:], in_=xt[:], identity=ident[:])
                nc.vector.tensor_copy(actT[:, 128 * t : 128 * (t + 1)], tp[:])

            for l in range(2):
                # ---- v' table: vloc[t] = f16(dinv_src * (W.T @ actT)[.,t].T)
                for (o, cw) in chunks:
                    vp = PAP.tile([128, 512], F32, tag="vp")
                    nc.tensor.matmul(out=vp[:, :cw], lhsT=w_sb[l][:],
                                     rhs=actT[:, o : o + cw],
                                     start=True, stop=True)
                    vt = PW.tile([128, 512], F16, tag="vt")
                    nc.vector.tensor_copy(vt[:, :cw], vp[:, :cw])
                    for s in range(0, cw, 128):
                        t = (o + s) // 128
                        tp = PT.tile([128, 128], F16, tag="tph")
                        nc.tensor.transpose(out=tp[:], in_=vt[:, s : s + 128],
                                            identity=ident16[:])
                        vv = PW.tile([128, 128], F16, tag="vv")
                        nc.vector.tensor_scalar(
                            vv[:], tp[:], dinv_sb[:, t : t + 1], None,
                            op0=mybir.AluOpType.mult)
                        nc.sync.dma_start(vloc_r[t], vv[:])

                # ---- halo exchange: AllGather the fp16 table
                nc.gpsimd.collective_compute(
                    "AllGather", mybir.AluOpType.bypass, replica_groups=rg,
                    ins=[vloc.ap().opt()], outs=[vfull.ap().opt()])

                # ---- gather + selector-matmul scatter, pair by pair
                call_tiles = [None] * (S // 128)
                band_next = [band_off[j] for j in range(NSLAB)]

                def issue_through(P):
                    # gather everything needed for pairs <= P, per band
                    for j in range(NSLAB):
                        limit = seg_off[j][P] + 128 * K[j][P]
                        while (band_next[j] < band_off[j + 1]
                               and band_next[j] < limit):
                            pos = band_next[j]
                            sz = min(CALLSZ, band_off[j + 1] - pos)
                            k = pos // 128          # chunk index of call
                            mt = PM.tile([128, CALLSZ // 128, 128], F16,
                                         tag="mt", name=f"mt{l}_{k}")
                            nc.gpsimd.dma_gather(
                                out_ap=mt[:, : sz // 128, :],
                                in_ap=vfull.ap()[j * SLAB :
                                                 min((j + 1) * SLAB,
                                                     NB * SH), :],
                                idxs_ap=gidx_sb[:, pos // 16 :
                                                (pos + sz) // 16],
                                num_idxs=sz, num_idxs_reg=sz,
                                elem_size=D, single_packet=False,
                                queue_num=qn[0] % 4)
                            qn[0] += 1
                            for cc in range(sz // 128):
                                call_tiles[k + cc] = (mt, cc)
                            band_next[j] += sz

                qn = [0]
                for P in range(NP):
                    issue_through(min(P + 2, NP - 1))
                    pcw = min(PAIR, SH - P * PAIR)
                    ps = PP.tile([128, PAIR], F32, tag="ps")
                    # chunk list for this pair across bands
                    pchunks = []
                    for j in range(NSLAB):
                        g0 = seg_off[j][P] // 128
                        pchunks.append((g0, K[j][P]))
                    total = sum(k for _, k in pchunks)
                    done = 0
                    for (g0, kk) in pchunks:
                        if kk == 0:
                            continue
                        sel = PSL.tile([128, kk, PAIR], F16, tag=f"sel{kk}",
                                       name=f"sel{P}_{kk}")
                        nc.vector.tensor_tensor(
                            out=sel[:],
                            in0=iota_sb[:].unsqueeze(1).broadcast_to(
                                [128, kk, PAIR]),
                            in1=dst_sb[:, g0 : g0 + kk].unsqueeze(2)
                                .broadcast_to([128, kk, PAIR]),
                            op=mybir.AluOpType.is_equal)
                        for i in range(kk):
                            g = g0 + i
                            mt, cc = call_tiles[g]
                            nc.tensor.matmul(
                                out=ps[:, :pcw],
                                lhsT=mt[:, cc, :],
                                rhs=sel[:, i, :pcw],
                                start=(done == 0), stop=(done == total - 1))
                            done += 1
                    # ---- conv = dinv*(ps + dinv*v); v recomputed on PE
                    vq = PV.tile([128, PAIR], F32, tag="vq")
                    nc.tensor.matmul(out=vq[:, :pcw], lhsT=w_sb[l][:],
                                     rhs=actT[:, P * PAIR : P * PAIR + pcw],
                                     start=True, stop=True)
                    dsl = dnr_sb[:, P * PAIR : P * PAIR + pcw]
                    t1 = PW.tile([128, PAIR], F32, tag="t1")
                    nc.vector.tensor_tensor(out=t1[:, :pcw], in0=vq[:, :pcw],
                                            in1=dsl, op=mybir.AluOpType.mult)
                    t2 = PW.tile([128, PAIR], F32, tag="t2")
                    nc.vector.tensor_tensor(out=t2[:, :pcw], in0=ps[:, :pcw],
                                            in1=t1[:, :pcw],
                                            op=mybir.AluOpType.add)
                    nc.vector.tensor_tensor(
                        out=actT[:, P * PAIR : P * PAIR + pcw],
                        in0=t2[:, :pcw], in1=dsl, op=mybir.AluOpType.mult)

                # ---- BN stats (biased, over the real N rows; pad rows are 0)
                nk = len(chunks)
                sumc = PS.tile([128, nk], F32, tag="sumc")
                sqc = PS.tile([128, nk], F32, tag="sqc")
                for k, (o, cw) in enumerate(chunks):
                    nc.vector.tensor_reduce(
                        out=sumc[:, k : k + 1], in_=actT[:, o : o + cw],
                        axis=mybir.AxisListType.X, op=mybir.AluOpType.add)
                    sq = PW.tile([128, 512], F32, tag="sq")
                    nc.scalar.activation(
                        out=sq[:, :cw], in_=actT[:, o : o + cw],
                        func=mybir.ActivationFunctionType.Square,
                        bias=zero_sb[:, 0:1],
                        accum_out=sqc[:, k : k + 1])
                stats_sb = PS.tile([128, 2], F32, tag="stats")
                nc.vector.tensor_reduce(out=stats_sb[:, 0:1], in_=sumc[:],
                                        axis=mybir.AxisListType.X,
                                        op=mybir.AluOpType.add)
                nc.vector.tensor_reduce(out=stats_sb[:, 1:2], in_=sqc[:],
                                        axis=mybir.AxisListType.X,
                                        op=mybir.AluOpType.add)
                nc.sync.dma_start(stats_in.ap(), stats_sb[:])
                nc.gpsimd.collective_compute(
                    "AllReduce", mybir.AluOpType.add, replica_groups=rg,
                    ins=[stats_in.ap().opt()], outs=[stats_out.ap().opt()])
                stats2 = PS.tile([128, 2], F32, tag="stats2")
                nc.sync.dma_start(stats2[:], stats_out.ap())

                # ---- BN affine params ([128,1] each)
                mu = PS.tile([128, 1], F32, tag="mu")
                nc.vector.tensor_scalar(mu[:], stats2[:, 0:1], 1.0 / N, None,
                                        op0=mybir.AluOpType.mult)
                e2 = PS.tile([128, 1], F32, tag="e2")
                nc.vector.tensor_scalar(e2[:], stats2[:, 1:2], 1.0 / N, None,
                                        op0=mybir.AluOpType.mult)
                var = PS.tile([128, 1], F32, tag="var")
                nc.vector.scalar_tensor_tensor(
                    out=var[:], in0=mu[:], scalar=-1.0, in1=mu[:],
                    op0=mybir.AluOpType.mult, op1=mybir.AluOpType.mult)
                nc.vector.tensor_tensor(out=var[:], in0=e2[:], in1=var[:],
                                        op=mybir.AluOpType.add)
                sd = PS.tile([128, 1], F32, tag="sd")
                nc.scalar.activation(out=sd[:], in_=var[:],
                                     func=mybir.ActivationFunctionType.Sqrt,
                                     bias=eps_sb[:, 0:1])
                rinv = PS.tile([128, 1], F32, tag="rinv")
                nc.vector.reciprocal(rinv[:], sd[:])
                alpha = PS.tile([128, 1], F32, tag="alpha")
                nc.vector.tensor_tensor(out=alpha[:], in0=gam_sb[l][:],
                                        in1=rinv[:], op=mybir.AluOpType.mult)
                bias_p = PS.tile([128, 1], F32, tag="biasp")
                nc.vector.scalar_tensor_tensor(
                    out=bias_p[:], in0=alpha[:], scalar=-1.0, in1=mu[:],
                    op0=mybir.AluOpType.mult, op1=mybir.AluOpType.mult)
                nc.vector.tensor_tensor(out=bias_p[:], in0=bet_sb[l][:],
                                        in1=bias_p[:], op=mybir.AluOpType.add)
                nalpha = PS.tile([128, 1], F32, tag="nalpha")
                nc.vector.tensor_scalar(nalpha[:], alpha[:], -1.0, None,
                                        op0=mybir.AluOpType.mult)
                nbias = PS.tile([128, 1], F32, tag="nbias")
                nc.vector.tensor_scalar(nbias[:], bias_p[:], -1.0, None,
                                        op0=mybir.AluOpType.mult)
                na = PS.tile([128, 1], F32, tag="na")
                nc.vector.tensor_scalar(na[:], a_sb[l][:], -1.0, None,
                                        op0=mybir.AluOpType.mult)

                # ---- fused BN + PReLU: y = relu(z) - a*relu(-z)
                for (o, cw) in chunks:
                    pos = PW.tile([128, 512], F32, tag="pos")
                    nc.scalar.activation(
                        out=pos[:, :cw], in_=actT[:, o : o + cw],
                        func=mybir.ActivationFunctionType.Relu,
                        bias=bias_p[:, :1], scale=alpha[:, :1])
                    neg = PW.tile([128, 512], F32, tag="neg")
                    nc.scalar.activation(
                        out=neg[:, :cw], in_=actT[:, o : o + cw],
                        func=mybir.ActivationFunctionType.Relu,
                        bias=nbias[:, :1], scale=nalpha[:, :1])
                    nc.vector.scalar_tensor_tensor(
                        out=actT[:, o : o + cw], in0=neg[:, :cw],
                        scalar=na[:, :1], in1=pos[:, :cw],
                        op0=mybir.AluOpType.mult, op1=mybir.AluOpType.add)

            # ---- write h2 back as [rows, feat]
            for t in range(NT):
                tp = PT.tile([128, 128], F32, tag="tp")
                nc.tensor.transpose(out=tp[:],
                                    in_=actT[:, 128 * t : 128 * (t + 1)],
                                    identity=ident[:])
                ot = PW.tile([128, 128], F32, tag="ot")
                nc.vector.tensor_copy(ot[:], tp[:])
                nc.sync.dma_start(out_r[t], ot[:])

    nc.compile()
    return nc


# ------------------------------------------------------------------- driver

_CACHE: dict = {}


def _get_compiled(key, N, SH, S, K, band_off, seg_off):
    if key not in _CACHE:
        nc = build_kernel(N, SH, S, K, band_off, seg_off)
        nc.m = get_hw_module(nc.m)
        _CACHE[key] = nc
    return _CACHE[key]


def make_in_maps(pre, w0, b0, gamma0, beta0, a0, w1, b1, gamma1, beta1, a1):
    def col(v):
        return np.ascontiguousarray(np.asarray(v, np.float32).reshape(-1, 1))

    def rep(v):
        return np.full((128, 1), np.float32(np.asarray(v).reshape(-1)[0]),
                       np.float32)

    iota = np.tile(np.arange(PAIR, dtype=np.float16)[None, :], (128, 1))
    maps = []
    for c in range(NB):
        maps.append({
            "x": pre["x_sh"][c],
            "gidx": pre["gidx"][c],
            "dst16": pre["dst16"][c],
            "dinvrow": pre["dinvrow"][c],
            "dinv_cols": pre["dinv_cols"][c],
            "iota256": iota,
            "w0": np.ascontiguousarray(np.asarray(w0, np.float32)),
            "w1": np.ascontiguousarray(np.asarray(w1, np.float32)),
            "gamma0": col(gamma0), "beta0": col(beta0), "a0": rep(a0),
            "gamma1": col(gamma1), "beta1": col(beta1), "a1": rep(a1),
        })
    return maps


def kernel(x, edge_index, w0, b0, gamma0, beta0, a0,
           w1, b1, gamma1, beta1, a1, _trace=False):
    x = np.asarray(x, np.float32)
    edge_index = np.asarray(edge_index, np.int64)
    pre = preprocess(x, edge_index)
    N, nsh, SH, S = pre["N"], pre["nsh"], pre["SH"], pre["S"]
    key = (N, SH, S, pre["K"], pre["band_off"])
    nc = _get_compiled(key, N, SH, S, pre["K"], pre["band_off"],
                       pre["seg_off"])
    in_maps = make_in_maps(pre, w0, b0, gamma0, beta0, a0,
                           w1, b1, gamma1, beta1, a1)
    res = bass_utils.run_bass_kernel_spmd(
        nc, in_maps, core_ids=list(range(NB)), trace=_trace)
    out = np.concatenate([res.results[c]["out"][:nsh] for c in range(NB)],
                         axis=0)[:N]
    if _trace:
        kernel.last_results = res
    return np.ascontiguousarray(out)
